# revision 2
# baseline (speedup 1.0000x reference)
"""Trainium2 Bass kernel for nn_AdaptiveSNN (B=128, T=32, D=6400, H=1000, A=4).

Strategy (data-parallel over batch, 8 NeuronCores, 16 batch rows each):

  The heavy layer-1 matmul h1[b,t,:] = x[b,t,:] @ W1.T is NOT sequential in t
  (the LIF recurrence only couples the cheap elementwise state update), so per
  core we compute H1 = X_local @ W1.T as one [512, 6400] x [6400, 1024] matmul
  (H padded 1000->1024), laid out transposed: psum banks hold H1.T chunks
  [128 H, 512 cols] where col = b*32 + t.

  - lhsT = W1.T tiles (host pre-transposed), rhs = X.T tiles (host
    pre-transposed), K = D on partitions, 50 k-tiles of 128.
  - m-outer loop (8 H-chunks); per chunk a K=1 "ones row" matmul folds b1 in
    exactly.
  - LIF1 runs per group of 2 H-chunks on DVE, overlapped with the next group's
    matmuls.  Per step t: acc = beta*mem + h (one scalar_tensor_tensor),
    mem = acc * keep (tensor_tensor), keep' = (mem <= 1) (tensor_scalar).
    keep' doubles as (1 - spk), stored for layer 2.
  - Layer 2: h2 = spk1 @ W2.T + b2 = (sum(W2)+b2) - keep1 @ W2.T, computed by
    accumulating (-W2).T @ KEEP1 chunks into one psum bank plus a K=1 ones-row
    with (sum(W2)+b2).  LIF2 on DVE, output spk2 = 1 - keep2.

  All matmuls in fp32 (fp32r was measured to have ~1e-3 error on HW; with only
  ~300 output spikes, a single threshold flip fails the rel-err gate).
"""

import numpy as np

import concourse.bass as bass
import concourse.tile as tile
from concourse import bacc, mybir
from concourse.bass_utils import run_bass_kernel_spmd

F32 = mybir.dt.float32
OP = mybir.AluOpType

NCORES = 8
B, T, D, H, A = 128, 32, 6400, 1000, 4
BL = B // NCORES            # 16 local batch
COLS = BL * T               # 512 matmul columns, col = b*32 + t
KT = D // 128               # 50 k tiles
HP = 1024                   # padded H
M = HP // 128               # 8 H-chunks
NGRP = 4                    # LIF1 groups of 2 chunks
BETA = 1.0 - 0.01
THRESH = 1.0

XCH = 5                     # x DMA chunks (10 k-tiles each)
XKT = KT // XCH
W1H = 2                     # w1 DMA halves per m-chunk (25 k-tiles each)
W1KT = KT // W1H

_CACHE = {}


def build():
    nc = bacc.Bacc("TRN2", target_bir_lowering=False, debug=False,
                   num_devices=NCORES)

    # host layouts (see kernel() for the exact host-side packing):
    #   xT   [128(p), KT, COLS]   xT[p,k,c]   = x_local[c//32, c%32, k*128+p]
    #   w1T  [M, 128(p), KT, 128] w1T[m,p,k,j] = W1p[m*128+j, k*128+p]
    #   b1r  [1, HP]
    #   w2n  [128(p), M*A]        w2n[p, m*4+a] = -W2p[a, m*128+p]
    #   s2b2 [1, A]               sum(W2p, axis=1) + b2
    xT_e = nc.declare_dram_parameter("xT", [128, KT, COLS], F32, isOutput=False)
    w1_e = nc.declare_dram_parameter("w1T", [M, 128, KT, 128], F32, isOutput=False)
    b1_e = nc.declare_dram_parameter("b1r", [1, HP], F32, isOutput=False)
    w2_e = nc.declare_dram_parameter("w2n", [128, M * A], F32, isOutput=False)
    s2_e = nc.declare_dram_parameter("s2b2", [1, A], F32, isOutput=False)
    out_e = nc.declare_dram_parameter("out", [A, COLS], F32, isOutput=True)

    with tile.TileContext(nc) as tc:
        with (
            tc.tile_pool(name="const", bufs=1) as cpool,
            tc.tile_pool(name="xsb", bufs=XCH) as xpool,
            tc.tile_pool(name="w1", bufs=3) as wpool,
            tc.tile_pool(name="h1g", bufs=2) as hpool,
            tc.tile_pool(name="keep", bufs=2) as kpool,
            tc.tile_pool(name="scratch", bufs=2) as spool,
            tc.tile_pool(name="ps1", bufs=7, space="PSUM") as ps1,
            tc.tile_pool(name="ps2", bufs=1, space="PSUM") as ps2,
        ):
            ones = cpool.tile([1, COLS], F32)
            nc.vector.memset(ones, 1.0)
            b1sb = cpool.tile([1, HP], F32)
            nc.sync.dma_start(out=b1sb, in_=b1_e.ap())
            w2sb = cpool.tile([128, M * A], F32)
            nc.sync.dma_start(out=w2sb, in_=w2_e.ap())
            s2sb = cpool.tile([1, A], F32)
            nc.sync.dma_start(out=s2sb, in_=s2_e.ap())

            mem1 = cpool.tile([128, M * BL], F32)
            nc.vector.memset(mem1, 0.0)
            mem1v = mem1.rearrange("p (m b) -> p m b", m=M)
            mem2 = cpool.tile([A, BL], F32)
            nc.vector.memset(mem2, 0.0)
            keep2 = cpool.tile([A, COLS], F32)
            k2v = keep2.rearrange("p (b t) -> p b t", b=BL)
            h2sb = cpool.tile([A, COLS], F32)
            h2v = h2sb.rearrange("p (b t) -> p b t", b=BL)
            spk2 = cpool.tile([A, COLS], F32)

            # x load: 5 chunks of 10 k-tiles
            xtiles = []
            for xc in range(XCH):
                xt = xpool.tile([128, XKT * COLS], F32, tag="x")
                nc.sync.dma_start(
                    out=xt, in_=xT_e.ap()[:, xc * XKT:(xc + 1) * XKT, :])
                xtiles.append(xt)

            def x_rhs(k):
                xt = xtiles[k // XKT]
                o = (k % XKT) * COLS
                return xt[:, o:o + COLS]

            # layer-2 bias/sum ones-row opens the psum2 accumulation group
            psum2 = ps2.tile([A, COLS], F32)
            nc.tensor.matmul(psum2, lhsT=s2sb, rhs=ones, start=True, stop=False)

            psum_tiles = [None] * M
            for m in range(M):
                psum = ps1.tile([128, COLS], F32, tag="ps1")
                psum_tiles[m] = psum
                # fold b1 in via a K=1 matmul
                nc.tensor.matmul(
                    psum, lhsT=b1sb[:, m * 128:(m + 1) * 128], rhs=ones,
                    start=True, stop=False)
                for h in range(W1H):
                    wt = wpool.tile([128, W1KT * 128], F32, tag="w1")
                    nc.sync.dma_start(
                        out=wt,
                        in_=w1_e.ap()[m, :, h * W1KT:(h + 1) * W1KT, :])
                    for kk in range(W1KT):
                        k = h * W1KT + kk
                        nc.tensor.matmul(
                            psum,
                            lhsT=wt[:, kk * 128:(kk + 1) * 128],
                            rhs=x_rhs(k),
                            start=False,
                            stop=(k == KT - 1))

                if m % 2 == 1:
                    g = m // 2
                    # evacuate the two banks, then LIF1 over 32 steps
                    h1g = hpool.tile([128, 2 * COLS], F32, tag="h1g")
                    for c in range(2):
                        nc.vector.tensor_copy(
                            h1g[:, c * COLS:(c + 1) * COLS],
                            psum_tiles[2 * g + c])
                    h4 = h1g.rearrange("p (c b t) -> p c b t", c=2, b=BL)
                    keepg = kpool.tile([128, 2 * COLS], F32, tag="keep")
                    k4 = keepg.rearrange("p (c b t) -> p c b t", c=2, b=BL)
                    memg = mem1v[:, 2 * g:2 * g + 2, :]
                    accg = spool.tile([128, 2 * BL], F32, tag="acc")
                    accv = accg.rearrange("p (c b) -> p c b", c=2)
                    for t in range(T):
                        if t == 0:
                            # mem=0, keep=1: mem <- h_0
                            nc.vector.scalar_tensor_tensor(
                                out=memg, in0=memg, scalar=BETA,
                                in1=h4[:, :, :, t], op0=OP.mult, op1=OP.add)
                        else:
                            nc.vector.scalar_tensor_tensor(
                                out=accv, in0=memg, scalar=BETA,
                                in1=h4[:, :, :, t], op0=OP.mult, op1=OP.add)
                            nc.vector.tensor_tensor(
                                out=memg, in0=accv, in1=k4[:, :, :, t - 1],
                                op=OP.mult)
                        nc.vector.tensor_scalar(
                            out=k4[:, :, :, t], in0=memg, scalar1=THRESH,
                            scalar2=None, op0=OP.is_le)
                    # accumulate layer-2 matmul for this group's two chunks
                    for c in range(2):
                        mm = 2 * g + c
                        nc.tensor.matmul(
                            psum2,
                            lhsT=w2sb[:, mm * A:(mm + 1) * A],
                            rhs=keepg[:, c * COLS:(c + 1) * COLS],
                            start=False,
                            stop=(mm == M - 1))

            # layer 2 LIF
            nc.vector.tensor_copy(h2sb, psum2)
            acc2 = cpool.tile([A, BL], F32)
            for t in range(T):
                if t == 0:
                    nc.vector.scalar_tensor_tensor(
                        out=mem2, in0=mem2, scalar=BETA, in1=h2v[:, :, t],
                        op0=OP.mult, op1=OP.add)
                else:
                    nc.vector.scalar_tensor_tensor(
                        out=acc2, in0=mem2, scalar=BETA, in1=h2v[:, :, t],
                        op0=OP.mult, op1=OP.add)
                    nc.vector.tensor_tensor(
                        out=mem2, in0=acc2, in1=k2v[:, :, t - 1], op=OP.mult)
                nc.vector.tensor_scalar(
                    out=k2v[:, :, t], in0=mem2, scalar1=THRESH,
                    scalar2=None, op0=OP.is_le)
            # spk2 = 1 - keep2
            nc.vector.tensor_scalar(
                out=spk2, in0=keep2, scalar1=-1.0, scalar2=1.0,
                op0=OP.mult, op1=OP.add)
            nc.sync.dma_start(out=out_e.ap(), in_=spk2)

    nc.compile()
    return nc


def _prep_shared(W1, b1, W2, b2):
    W1p = np.zeros((HP, D), np.float32)
    W1p[:H] = W1
    # w1T[m,p,k,j] = W1p[m*128+j, k*128+p]
    w1T = np.ascontiguousarray(
        W1p.reshape(M, 128, KT, 128).transpose(0, 3, 2, 1))
    b1p = np.zeros((1, HP), np.float32)
    b1p[0, :H] = b1
    W2p = np.zeros((A, HP), np.float32)
    W2p[:, :H] = W2
    # w2n[p, m*4+a] = -W2p[a, m*128+p]
    w2n = np.ascontiguousarray((-W2p).reshape(A, M, 128).transpose(2, 1, 0)
                               .reshape(128, M * A))
    s2b2 = (W2p.sum(axis=1, dtype=np.float32) + b2).reshape(1, A)
    s2b2 = np.ascontiguousarray(s2b2.astype(np.float32))
    return w1T, b1p, w2n, s2b2


def _prep_x(x, c):
    xs = x[c * BL:(c + 1) * BL].reshape(COLS, D)       # row = b*32+t
    xT = np.ascontiguousarray(xs.T)                    # [D, COLS]
    # [128(p), KT, COLS]: xT3[p,k,c] = xT[k*128+p, c]
    return np.ascontiguousarray(xT.reshape(KT, 128, COLS).transpose(1, 0, 2))


def kernel(x, W1, b1, W2, b2, _want_results=False):
    x = np.ascontiguousarray(x, np.float32)
    W1 = np.asarray(W1, np.float32)
    b1 = np.asarray(b1, np.float32)
    W2 = np.asarray(W2, np.float32)
    b2 = np.asarray(b2, np.float32)

    if "nc" not in _CACHE:
        _CACHE["nc"] = build()
    nc = _CACHE["nc"]

    w1T, b1p, w2n, s2b2 = _prep_shared(W1, b1, W2, b2)
    in_maps = []
    for c in range(NCORES):
        in_maps.append({
            "xT": _prep_x(x, c),
            "w1T": w1T,
            "b1r": b1p,
            "w2n": w2n,
            "s2b2": s2b2,
        })

    res = run_bass_kernel_spmd(nc, in_maps, core_ids=list(range(NCORES)))

    out = np.empty((B, T, A), np.float32)
    for c in range(NCORES):
        o = res.results[c]["out"]                      # [A, COLS]
        out[c * BL:(c + 1) * BL] = o.T.reshape(BL, T, A)
    if _want_results:
        return out, res
    return out


# revision 5
# speedup vs baseline: 1.4667x; 1.4667x over previous
"""Trainium2 Bass kernel for nn_AdaptiveSNN (B=128, T=32, D=6400, H=1000, A=4).

Strategy (data-parallel over batch, 8 NeuronCores, 16 batch rows each):

  The heavy layer-1 matmul h1[b,t,:] = x[b,t,:] @ W1.T is NOT sequential in t
  (the LIF recurrence only couples the cheap elementwise state update), so per
  core we compute H1 = X_local @ W1.T as one [512, 6400] x [6400, 1024] matmul
  (H padded 1000->1024), laid out transposed: psum banks hold H1.T chunks
  [128 H, 512 cols] with col = t*16 + b (t-major, so per-step LIF slices are
  contiguous 16-element runs and layer-2 column ranges by time are contiguous).

  fp16 hi/lo x3 matmul: fp32 operands are split a = ah + al with ah = fp16(a),
  al = fp16((a - ah) * 2^12); the product needs ah*bh (psum bank HI) and
  ah*bl + al*bh (psum bank LO, uniformly scaled 2^12); al*bl (~2^-24 relative)
  is dropped.  h = HI + 2^-12 * LO then matches an fp32 matmul up to normal
  fp32 rounding (fp16 products are exact in fp32; PSUM accumulates fp32).
  W1 is pre-scaled by 256 so its lo-part stays in fp16 normal range; the LIF
  recurrence is scale-invariant, so mem1 simply runs at 256x with threshold
  256 (exact powers of two).  fp16 streams 1 cycle/row through the PE vs ~6
  effective for fp32 (measured 710ns per half-pass at N=512 even warm).

  - lhsT = W1.T tiles (host pre-transposed), rhs = X.T tiles (host
    pre-transposed), K = D on partitions, 50 k-tiles of 128.
  - m-outer loop (8 H-chunks); K=1 "ones row" matmuls fold b1 in exactly
    (hi/lo split as well).
  - LIF1 runs per group of 2 H-chunks on DVE, overlapped with the next group's
    matmuls.  Per step t: acc = beta*mem + h (scalar_tensor_tensor),
    mem = acc * keep (tensor_tensor), keep' = (mem <= thresh) (tensor_scalar).
    keep' doubles as (1 - spk), stored for layer 2.
  - Layer 2: h2 = spk1 @ W2.T + b2 = (sum(W2)+b2) - keep1 @ W2.T, computed by
    accumulating (-W2).T @ KEEP1 group tiles into one psum bank (fp32 matmuls,
    tiny) plus a K=1 ones-row with (sum(W2)+b2).  LIF2 on DVE at the end.
  - Output spk2 = 1 - keep2, written as [A, 512]; host transposes back.

  (fp32r was measured at ~1e-3 error on HW; with only ~300 output spikes a
  single threshold flip fails the rel-err gate, so only fp32-grade math is
  usable.)
"""

import numpy as np

import concourse.bass as bass
import concourse.tile as tile
from concourse import bacc, mybir
from concourse.bass_utils import run_bass_kernel_spmd

F32 = mybir.dt.float32
F16 = mybir.dt.float16
OP = mybir.AluOpType

NCORES = 8
B, T, D, H, A = 128, 32, 6400, 1000, 4
BL = B // NCORES            # 16 local batch
COLS = BL * T               # 512 matmul columns, col = t*16 + b (t-major)
KT = D // 128               # 50 k tiles
HP = 1024                   # padded H
M = HP // 128               # 8 H-chunks
BETA = 1.0 - 0.01

WSCALE = 256.0              # W1 pre-scale (exact power of 2)
LSCALE = 4096.0             # lo-part scale 2^12

# FP16X3 True: hi/lo fp16 3-pass matmul.  False: plain fp32 matmul.
FP16X3 = True

XCH = 5                     # x DMA chunks (10 k-tiles each)
XKT = KT // XCH
W1H = 2                     # w1 DMA halves per m-chunk (25 k-tiles each)
W1KT = KT // W1H

_CACHE = {}


def _lif_steps(nc, memv, accv, h4, k4, thresh):
    """Emit the 32-step LIF recurrence.

    memv/accv: [p, ..., b] fp32 SBUF views; h4/k4: [p, ..., b, t] views.
    keep column t holds (mem_t <= thresh) = 1 - spk_t.
    """
    for t in range(T):
        if t == 0:
            # mem=0, keep=1: mem <- h_0  (beta*0 + h)
            nc.vector.scalar_tensor_tensor(
                out=memv, in0=memv, scalar=BETA,
                in1=h4[..., 0], op0=OP.mult, op1=OP.add)
        else:
            nc.vector.scalar_tensor_tensor(
                out=accv, in0=memv, scalar=BETA,
                in1=h4[..., t], op0=OP.mult, op1=OP.add)
            nc.vector.tensor_tensor(
                out=memv, in0=accv, in1=k4[..., t - 1], op=OP.mult)
        nc.vector.tensor_scalar(
            out=k4[..., t], in0=memv, scalar1=thresh,
            scalar2=None, op0=OP.is_le)


def build():
    nc = bacc.Bacc("TRN2", target_bir_lowering=False, debug=False,
                   num_devices=NCORES)

    MMDT = F16 if FP16X3 else F32
    THR1 = 1.0 * WSCALE if FP16X3 else 1.0

    # host layouts (see kernel() for the exact host-side packing):
    #   xh/xl [128(p), KT, COLS]      x.T tiles, col = t*16+b, hi/lo fp16
    #   w1h/w1l [M, 128(p), KT, 128]  (256*W1).T tiles, hi/lo fp16
    #   b1h/b1l [1, HP]               256*b1 hi/lo rows
    #   w2n  [128(p), M*A]            w2n[p, m*4+a] = -W2p[a, m*128+p] (fp32)
    #   s2b2 [1, A]                   sum(W2p, axis=1) + b2 (fp32)
    xh_e = nc.declare_dram_parameter("xh", [128, KT, COLS], MMDT, isOutput=False)
    w1h_e = nc.declare_dram_parameter("w1h", [M, 128, KT, 128], MMDT, isOutput=False)
    b1h_e = nc.declare_dram_parameter("b1h", [1, HP], MMDT, isOutput=False)
    if FP16X3:
        xl_e = nc.declare_dram_parameter("xl", [128, KT, COLS], F16, isOutput=False)
        w1l_e = nc.declare_dram_parameter("w1l", [M, 128, KT, 128], F16, isOutput=False)
        b1l_e = nc.declare_dram_parameter("b1l", [1, HP], F16, isOutput=False)
    w2_e = nc.declare_dram_parameter("w2n", [128, M * A], F32, isOutput=False)
    s2_e = nc.declare_dram_parameter("s2b2", [1, A], F32, isOutput=False)
    out_e = nc.declare_dram_parameter("out", [A, COLS], F32, isOutput=True)

    with tile.TileContext(nc) as tc:
        with (
            tc.tile_pool(name="const", bufs=1) as cpool,
            tc.tile_pool(name="xsb", bufs=(2 * XCH if FP16X3 else XCH)) as xpool,
            tc.tile_pool(name="w1", bufs=(8 if FP16X3 else 4)) as wpool,
            tc.tile_pool(name="h1g", bufs=2) as hpool,
            tc.tile_pool(name="keep", bufs=2) as kpool,
            tc.tile_pool(name="scratch", bufs=2) as spool,
            tc.tile_pool(name="ps1", bufs=(6 if FP16X3 else 7), space="PSUM") as ps1,
            tc.tile_pool(name="ps2", bufs=1, space="PSUM") as ps2,
        ):
            ones = cpool.tile([1, COLS], MMDT)
            nc.vector.memset(ones, 1.0)
            ones32 = cpool.tile([1, COLS], F32)
            nc.vector.memset(ones32, 1.0)
            b1h = cpool.tile([1, HP], MMDT)
            nc.sync.dma_start(out=b1h, in_=b1h_e.ap())
            if FP16X3:
                b1l = cpool.tile([1, HP], F16)
                nc.sync.dma_start(out=b1l, in_=b1l_e.ap())
            w2sb = cpool.tile([128, M * A], F32)
            nc.sync.dma_start(out=w2sb, in_=w2_e.ap())
            s2sb = cpool.tile([1, A], F32)
            nc.sync.dma_start(out=s2sb, in_=s2_e.ap())

            mem1 = cpool.tile([128, M * BL], F32)
            nc.vector.memset(mem1, 0.0)
            mem1v = mem1.rearrange("p (m b) -> p m b", m=M)
            mem2 = cpool.tile([A, BL], F32)
            nc.vector.memset(mem2, 0.0)
            keep2 = cpool.tile([A, COLS], F32)
            k2v = keep2.rearrange("p (t b) -> p b t", t=T)
            h2sb = cpool.tile([A, COLS], F32)
            h2v = h2sb.rearrange("p (t b) -> p b t", t=T)
            spk2 = cpool.tile([A, COLS], F32)
            acc2 = cpool.tile([A, BL], F32)

            # x load; chunk 0 split finer so the PE can start sooner
            xparams = [xh_e, xl_e] if FP16X3 else [xh_e]
            xtiles = [[] for _ in xparams]
            for xc in range(XCH):
                for xi, xe in enumerate(xparams):
                    xt = xpool.tile([128, XKT * COLS], MMDT, tag="x")
                    if xc == 0:
                        half = XKT // 2
                        nc.sync.dma_start(
                            out=xt[:, :half * COLS], in_=xe.ap()[:, :half, :])
                        nc.sync.dma_start(
                            out=xt[:, half * COLS:],
                            in_=xe.ap()[:, half:XKT, :])
                    else:
                        nc.sync.dma_start(
                            out=xt, in_=xe.ap()[:, xc * XKT:(xc + 1) * XKT, :])
                    xtiles[xi].append(xt)

            def x_rhs(xi, k):
                xt = xtiles[xi][k // XKT]
                o = (k % XKT) * COLS
                return xt[:, o:o + COLS]

            # layer-2 bias/sum ones-row opens the psum2 accumulation group
            psum2 = ps2.tile([A, COLS], F32)
            nc.tensor.matmul(psum2, lhsT=s2sb, rhs=ones32, start=True, stop=False)

            wparams = [w1h_e, w1l_e] if FP16X3 else [w1h_e]
            h1g = None
            for m in range(M):
                ph = ps1.tile([128, COLS], F32, tag="ps1")
                nc.tensor.matmul(
                    ph, lhsT=b1h[:, m * 128:(m + 1) * 128], rhs=ones,
                    start=True, stop=False)
                if FP16X3:
                    pl = ps1.tile([128, COLS], F32, tag="ps1")
                    nc.tensor.matmul(
                        pl, lhsT=b1l[:, m * 128:(m + 1) * 128], rhs=ones,
                        start=True, stop=False)
                for hf in range(W1H):
                    wts = []
                    for we in wparams:
                        wt = wpool.tile([128, W1KT * 128], MMDT, tag="w1")
                        # first half of m=0 arrives in 5-k-tile pieces so the
                        # PE can start ~4x sooner
                        nq = 5 if (m == 0 and hf == 0) else 1
                        step = W1KT // nq
                        for q in range(nq):
                            nc.sync.dma_start(
                                out=wt[:, q * step * 128:(q + 1) * step * 128],
                                in_=we.ap()[m, :, hf * W1KT + q * step:
                                            hf * W1KT + (q + 1) * step, :])
                        wts.append(wt)
                    for kk in range(W1KT):
                        k = hf * W1KT + kk
                        last = (k == KT - 1)
                        sl = slice(kk * 128, (kk + 1) * 128)
                        # hi*hi -> HI bank; hi*lo + lo*hi -> LO bank
                        nc.tensor.matmul(
                            ph, lhsT=wts[0][:, sl], rhs=x_rhs(0, k),
                            start=False, stop=last)
                        if FP16X3:
                            nc.tensor.matmul(
                                pl, lhsT=wts[0][:, sl], rhs=x_rhs(1, k),
                                start=False, stop=False)
                            nc.tensor.matmul(
                                pl, lhsT=wts[1][:, sl], rhs=x_rhs(0, k),
                                start=False, stop=last)

                # evacuate: h = HI + 2^-12 * LO  (h stays at 256*h1 scale)
                c = m % 2
                if c == 0:
                    h1g = hpool.tile([128, 2 * COLS], F32, tag="h1g")
                hslc = h1g[:, c * COLS:(c + 1) * COLS]
                nc.vector.tensor_copy(hslc, ph)
                if FP16X3:
                    nc.vector.scalar_tensor_tensor(
                        out=hslc, in0=pl, scalar=1.0 / LSCALE, in1=hslc,
                        op0=OP.mult, op1=OP.add)

                if c == 1:
                    g = m // 2
                    h4 = h1g.rearrange("p (c t b) -> p c b t", c=2, t=T)
                    keepg = kpool.tile([128, 2 * COLS], F32, tag="keep")
                    k4 = keepg.rearrange("p (c t b) -> p c b t", c=2, t=T)
                    memv = mem1v[:, 2 * g:2 * g + 2, :]
                    accg = spool.tile([128, 2 * BL], F32, tag="acc")
                    accv = accg.rearrange("p (c b) -> p c b", c=2)
                    _lif_steps(nc, memv, accv, h4, k4, THR1)
                    for cc in range(2):
                        mm = 2 * g + cc
                        nc.tensor.matmul(
                            psum2,
                            lhsT=w2sb[:, mm * A:(mm + 1) * A],
                            rhs=keepg[:, cc * COLS:(cc + 1) * COLS],
                            start=False, stop=(mm == M - 1))

            # layer 2 LIF
            nc.vector.tensor_copy(h2sb, psum2)
            _lif_steps(nc, mem2, acc2, h2v, k2v, 1.0)
            # spk2 = 1 - keep2
            nc.vector.tensor_scalar(
                out=spk2, in0=keep2, scalar1=-1.0, scalar2=1.0,
                op0=OP.mult, op1=OP.add)
            nc.sync.dma_start(out=out_e.ap(), in_=spk2)

    nc.compile()
    return nc


def _split16(a):
    """fp32 array -> (hi, lo) fp16 with lo scaled by 2^12."""
    hi = a.astype(np.float16)
    lo = ((a - hi.astype(np.float32)) * LSCALE).astype(np.float16)
    return hi, lo


def _prep_shared(W1, b1, W2, b2):
    W1p = np.zeros((HP, D), np.float32)
    W1p[:H] = W1
    b1p = np.zeros((1, HP), np.float32)
    b1p[0, :H] = b1
    if FP16X3:
        W1p *= WSCALE
        b1p = b1p * WSCALE
    # w1T[m,p,k,j] = W1p[m*128+j, k*128+p]
    w1T = np.ascontiguousarray(
        W1p.reshape(M, 128, KT, 128).transpose(0, 3, 2, 1))
    W2p = np.zeros((A, HP), np.float32)
    W2p[:, :H] = W2
    # w2n[p, m*4+a] = -W2p[a, m*128+p]
    w2n = np.ascontiguousarray((-W2p).reshape(A, M, 128).transpose(2, 1, 0)
                               .reshape(128, M * A))
    s2b2 = (W2p.sum(axis=1, dtype=np.float32) + b2).reshape(1, A)
    s2b2 = np.ascontiguousarray(s2b2.astype(np.float32))
    shared = {"w2n": w2n, "s2b2": s2b2}
    if FP16X3:
        shared["w1h"], shared["w1l"] = _split16(w1T)
        shared["b1h"], shared["b1l"] = _split16(b1p)
    else:
        shared["w1h"] = w1T
        shared["b1h"] = b1p
    return shared


def _prep_x(x, c):
    # rows t-major: row = t*16 + b
    xs = np.ascontiguousarray(
        x[c * BL:(c + 1) * BL].transpose(1, 0, 2)).reshape(COLS, D)
    xT = np.ascontiguousarray(xs.T)                    # [D, COLS]
    # [128(p), KT, COLS]: xT3[p,k,c] = xT[k*128+p, c]
    xT3 = np.ascontiguousarray(xT.reshape(KT, 128, COLS).transpose(1, 0, 2))
    if FP16X3:
        hi, lo = _split16(xT3)
        return {"xh": hi, "xl": lo}
    return {"xh": xT3}


def kernel(x, W1, b1, W2, b2, _want_results=False):
    x = np.ascontiguousarray(np.asarray(x), np.float32)
    W1 = np.asarray(W1, np.float32)
    b1 = np.asarray(b1, np.float32)
    W2 = np.asarray(W2, np.float32)
    b2 = np.asarray(b2, np.float32)

    if "nc" not in _CACHE:
        _CACHE["nc"] = build()
    nc = _CACHE["nc"]

    shared = _prep_shared(W1, b1, W2, b2)
    in_maps = []
    for c in range(NCORES):
        m = dict(shared)
        m.update(_prep_x(x, c))
        in_maps.append(m)

    res = run_bass_kernel_spmd(nc, in_maps, core_ids=list(range(NCORES)))

    out = np.empty((B, T, A), np.float32)
    for c in range(NCORES):
        o = res.results[c]["out"]                      # [A, COLS], col=t*16+b
        out[c * BL:(c + 1) * BL] = o.T.reshape(T, BL, A).transpose(1, 0, 2)
    if _want_results:
        return out, res
    return out


# revision 7
# speedup vs baseline: 1.5661x; 1.0678x over previous
"""Trainium2 Bass kernel for nn_AdaptiveSNN (B=128, T=32, D=6400, H=1000, A=4).

Strategy (data-parallel over batch, 8 NeuronCores, 16 batch rows each):

  The heavy layer-1 matmul h1[b,t,:] = x[b,t,:] @ W1.T is NOT sequential in t
  (the LIF recurrence only couples the cheap elementwise state update), so per
  core we compute H1 = X_local @ W1.T as one [512, 6400] x [6400, 1024] matmul
  (H padded 1000->1024), laid out transposed: psum banks hold H1.T chunks
  [128 H, 512 cols] with col = t*16 + b (t-major, so per-step LIF slices are
  contiguous 16-element runs and layer-2 column ranges by time are contiguous).

  fp16 hi/lo x3 matmul: fp32 operands are split a = ah + al with ah = fp16(a),
  al = fp16((a - ah) * 2^12); the product needs ah*bh (psum bank HI) and
  ah*bl + al*bh (psum bank LO, uniformly scaled 2^12); al*bl (~2^-24 relative)
  is dropped.  h = HI + 2^-12 * LO then matches an fp32 matmul up to normal
  fp32 rounding (fp16 products are exact in fp32; PSUM accumulates fp32).
  W1 is pre-scaled by 256 so its lo-part stays in fp16 normal range; the LIF
  recurrence is scale-invariant, so mem1 simply runs at 256x with threshold
  256 (exact powers of two).  fp16 streams 1 cycle/row through the PE vs ~6
  effective for fp32 (measured 710ns per half-pass at N=512 even warm).

  - lhsT = W1.T tiles (host pre-transposed), rhs = X.T tiles (host
    pre-transposed), K = D on partitions, 50 k-tiles of 128.
  - m-outer loop (8 H-chunks); K=1 "ones row" matmuls fold b1 in exactly
    (hi/lo split as well).
  - LIF1 runs per group of 2 H-chunks on DVE, overlapped with the next group's
    matmuls.  Per step t: acc = beta*mem + h (scalar_tensor_tensor),
    mem = acc * keep (tensor_tensor), keep' = (mem <= thresh) (tensor_scalar).
    keep' doubles as (1 - spk), stored for layer 2.
  - Layer 2: h2 = spk1 @ W2.T + b2 = (sum(W2)+b2) - keep1 @ W2.T, computed by
    accumulating (-W2).T @ KEEP1 group tiles into one psum bank (fp32 matmuls,
    tiny) plus a K=1 ones-row with (sum(W2)+b2).  LIF2 on DVE at the end.
  - Output spk2 = 1 - keep2, written as [A, 512]; host transposes back.

  (fp32r was measured at ~1e-3 error on HW; with only ~300 output spikes a
  single threshold flip fails the rel-err gate, so only fp32-grade math is
  usable.)
"""

import numpy as np

import concourse.bass as bass
import concourse.tile as tile
from concourse import bacc, mybir
from concourse.bass_utils import run_bass_kernel_spmd

F32 = mybir.dt.float32
F16 = mybir.dt.float16
OP = mybir.AluOpType

NCORES = 8
B, T, D, H, A = 128, 32, 6400, 1000, 4
BL = B // NCORES            # 16 local batch
COLS = BL * T               # 512 matmul columns, col = t*16 + b (t-major)
KT = D // 128               # 50 k tiles
HP = 1024                   # padded H
M = HP // 128               # 8 H-chunks
BETA = 1.0 - 0.01

WSCALE = 256.0              # W1 pre-scale (exact power of 2)
LSCALE = 4096.0             # lo-part scale 2^12

# FP16X3 True: hi/lo fp16 3-pass matmul.  False: plain fp32 matmul.
FP16X3 = True

XCH = 5                     # x DMA chunks (10 k-tiles each)
XKT = KT // XCH
W1H = 2                     # w1 DMA halves per m-chunk (25 k-tiles each)
W1KT = KT // W1H

_CACHE = {}


def _lif_steps(nc, memv, accv, h4, k4, thresh):
    """Emit the 32-step LIF recurrence.

    memv/accv: [p, ..., b] fp32 SBUF views; h4/k4: [p, ..., b, t] views.
    keep column t holds (mem_t <= thresh) = 1 - spk_t.
    """
    for t in range(T):
        if t == 0:
            # mem=0, keep=1: mem <- h_0  (beta*0 + h)
            nc.vector.scalar_tensor_tensor(
                out=memv, in0=memv, scalar=BETA,
                in1=h4[..., 0], op0=OP.mult, op1=OP.add)
        else:
            nc.vector.scalar_tensor_tensor(
                out=accv, in0=memv, scalar=BETA,
                in1=h4[..., t], op0=OP.mult, op1=OP.add)
            nc.vector.tensor_tensor(
                out=memv, in0=accv, in1=k4[..., t - 1], op=OP.mult)
        nc.vector.tensor_scalar(
            out=k4[..., t], in0=memv, scalar1=thresh,
            scalar2=None, op0=OP.is_le)


def build():
    nc = bacc.Bacc("TRN2", target_bir_lowering=False, debug=False,
                   num_devices=NCORES)

    MMDT = F16 if FP16X3 else F32
    THR1 = 1.0 * WSCALE if FP16X3 else 1.0

    # host layouts (see kernel() for the exact host-side packing):
    #   xh/xl [128(p), KT, COLS]      x.T tiles, col = t*16+b, hi/lo fp16
    #   w1h/w1l [M, 128(p), KT, 128]  (256*W1).T tiles, hi/lo fp16
    #   b1h/b1l [1, HP]               256*b1 hi/lo rows
    #   w2n  [128(p), M*A]            w2n[p, m*4+a] = -W2p[a, m*128+p] (fp32)
    #   s2b2 [1, A]                   sum(W2p, axis=1) + b2 (fp32)
    xh_e = nc.declare_dram_parameter("xh", [128, KT, COLS], MMDT, isOutput=False)
    w1h_e = nc.declare_dram_parameter("w1h", [M, 128, KT, 128], MMDT, isOutput=False)
    b1h_e = nc.declare_dram_parameter("b1h", [1, HP], MMDT, isOutput=False)
    if FP16X3:
        xl_e = nc.declare_dram_parameter("xl", [128, KT, COLS], F16, isOutput=False)
        w1l_e = nc.declare_dram_parameter("w1l", [M, 128, KT, 128], F16, isOutput=False)
        b1l_e = nc.declare_dram_parameter("b1l", [1, HP], F16, isOutput=False)
    w2_e = nc.declare_dram_parameter("w2n", [128, M * A], F32, isOutput=False)
    s2_e = nc.declare_dram_parameter("s2b2", [1, A], F32, isOutput=False)
    out_e = nc.declare_dram_parameter("out", [A, COLS], F32, isOutput=True)

    with tile.TileContext(nc) as tc:
        with (
            tc.tile_pool(name="const", bufs=1) as cpool,
            tc.tile_pool(name="xsb", bufs=(2 * XCH if FP16X3 else XCH)) as xpool,
            tc.tile_pool(name="w1", bufs=(8 if FP16X3 else 4)) as wpool,
            tc.tile_pool(name="h1g", bufs=2) as hpool,
            tc.tile_pool(name="keep", bufs=2) as kpool,
            tc.tile_pool(name="scratch", bufs=2) as spool,
            tc.tile_pool(name="ps1", bufs=(6 if FP16X3 else 7), space="PSUM") as ps1,
            tc.tile_pool(name="ps2", bufs=1, space="PSUM") as ps2,
        ):
            # Small constants + x go on the Scalar HWDGE queue; W1 streams on
            # the Sync HWDGE queue.  Two independent FIFOs -> W1's first tiles
            # aren't stuck behind 13MB of x (measured 38us PE stall).
            ones = cpool.tile([1, COLS], MMDT)
            nc.vector.memset(ones, 1.0)
            ones32 = cpool.tile([1, COLS], F32)
            nc.vector.memset(ones32, 1.0)
            b1h = cpool.tile([1, HP], MMDT)
            nc.scalar.dma_start(out=b1h, in_=b1h_e.ap())
            if FP16X3:
                b1l = cpool.tile([1, HP], F16)
                nc.scalar.dma_start(out=b1l, in_=b1l_e.ap())
            w2sb = cpool.tile([128, M * A], F32)
            nc.scalar.dma_start(out=w2sb, in_=w2_e.ap())
            s2sb = cpool.tile([1, A], F32)
            nc.scalar.dma_start(out=s2sb, in_=s2_e.ap())

            mem1 = cpool.tile([128, M * BL], F32)
            nc.vector.memset(mem1, 0.0)
            mem1v = mem1.rearrange("p (m b) -> p m b", m=M)
            mem2 = cpool.tile([A, BL], F32)
            nc.vector.memset(mem2, 0.0)
            keep2 = cpool.tile([A, COLS], F32)
            k2v = keep2.rearrange("p (t b) -> p b t", t=T)
            h2sb = cpool.tile([A, COLS], F32)
            h2v = h2sb.rearrange("p (t b) -> p b t", t=T)
            spk2 = cpool.tile([A, COLS], F32)
            acc2 = cpool.tile([A, BL], F32)

            # x load; chunk 0 split finer so the PE can start sooner
            xparams = [xh_e, xl_e] if FP16X3 else [xh_e]
            xtiles = [[] for _ in xparams]
            for xc in range(XCH):
                for xi, xe in enumerate(xparams):
                    xt = xpool.tile([128, XKT * COLS], MMDT, tag="x")
                    if xc == 0:
                        half = XKT // 2
                        nc.scalar.dma_start(
                            out=xt[:, :half * COLS], in_=xe.ap()[:, :half, :])
                        nc.scalar.dma_start(
                            out=xt[:, half * COLS:],
                            in_=xe.ap()[:, half:XKT, :])
                    else:
                        nc.scalar.dma_start(
                            out=xt, in_=xe.ap()[:, xc * XKT:(xc + 1) * XKT, :])
                    xtiles[xi].append(xt)

            def x_rhs(xi, k):
                xt = xtiles[xi][k // XKT]
                o = (k % XKT) * COLS
                return xt[:, o:o + COLS]

            # layer-2 bias/sum ones-row opens the psum2 accumulation group
            psum2 = ps2.tile([A, COLS], F32)
            nc.tensor.matmul(psum2, lhsT=s2sb, rhs=ones32, start=True, stop=False)

            wparams = [w1h_e, w1l_e] if FP16X3 else [w1h_e]
            h1g = None
            for m in range(M):
                ph = ps1.tile([128, COLS], F32, tag="ps1")
                nc.tensor.matmul(
                    ph, lhsT=b1h[:, m * 128:(m + 1) * 128], rhs=ones,
                    start=True, stop=False)
                if FP16X3:
                    pl = ps1.tile([128, COLS], F32, tag="ps1")
                    nc.tensor.matmul(
                        pl, lhsT=b1l[:, m * 128:(m + 1) * 128], rhs=ones,
                        start=True, stop=False)
                for hf in range(W1H):
                    wts = []
                    for we in wparams:
                        wt = wpool.tile([128, W1KT * 128], MMDT, tag="w1")
                        # first half of m=0 arrives in 5-k-tile pieces so the
                        # PE can start ~4x sooner
                        nq = 5 if (m == 0 and hf == 0) else 1
                        step = W1KT // nq
                        for q in range(nq):
                            nc.sync.dma_start(
                                out=wt[:, q * step * 128:(q + 1) * step * 128],
                                in_=we.ap()[m, :, hf * W1KT + q * step:
                                            hf * W1KT + (q + 1) * step, :])
                        wts.append(wt)
                    for kk in range(W1KT):
                        k = hf * W1KT + kk
                        last = (k == KT - 1)
                        sl = slice(kk * 128, (kk + 1) * 128)
                        # hi*hi -> HI bank; hi*lo + lo*hi -> LO bank
                        nc.tensor.matmul(
                            ph, lhsT=wts[0][:, sl], rhs=x_rhs(0, k),
                            start=False, stop=last)
                        if FP16X3:
                            nc.tensor.matmul(
                                pl, lhsT=wts[0][:, sl], rhs=x_rhs(1, k),
                                start=False, stop=False)
                            nc.tensor.matmul(
                                pl, lhsT=wts[1][:, sl], rhs=x_rhs(0, k),
                                start=False, stop=last)

                # evacuate: h = HI + 2^-12 * LO  (h stays at 256*h1 scale)
                c = m % 2
                if c == 0:
                    h1g = hpool.tile([128, 2 * COLS], F32, tag="h1g")
                hslc = h1g[:, c * COLS:(c + 1) * COLS]
                nc.vector.tensor_copy(hslc, ph)
                if FP16X3:
                    nc.vector.scalar_tensor_tensor(
                        out=hslc, in0=pl, scalar=1.0 / LSCALE, in1=hslc,
                        op0=OP.mult, op1=OP.add)

                if c == 1:
                    g = m // 2
                    h4 = h1g.rearrange("p (c t b) -> p c b t", c=2, t=T)
                    keepg = kpool.tile([128, 2 * COLS], F32, tag="keep")
                    k4 = keepg.rearrange("p (c t b) -> p c b t", c=2, t=T)
                    memv = mem1v[:, 2 * g:2 * g + 2, :]
                    accg = spool.tile([128, 2 * BL], F32, tag="acc")
                    accv = accg.rearrange("p (c b) -> p c b", c=2)
                    _lif_steps(nc, memv, accv, h4, k4, THR1)
                    for cc in range(2):
                        mm = 2 * g + cc
                        nc.tensor.matmul(
                            psum2,
                            lhsT=w2sb[:, mm * A:(mm + 1) * A],
                            rhs=keepg[:, cc * COLS:(cc + 1) * COLS],
                            start=False, stop=(mm == M - 1))

            # layer 2 LIF
            nc.vector.tensor_copy(h2sb, psum2)
            _lif_steps(nc, mem2, acc2, h2v, k2v, 1.0)
            # spk2 = 1 - keep2
            nc.vector.tensor_scalar(
                out=spk2, in0=keep2, scalar1=-1.0, scalar2=1.0,
                op0=OP.mult, op1=OP.add)
            nc.sync.dma_start(out=out_e.ap(), in_=spk2)

    nc.compile()
    return nc


def _split16(a):
    """fp32 array -> (hi, lo) fp16 with lo scaled by 2^12."""
    hi = a.astype(np.float16)
    lo = ((a - hi.astype(np.float32)) * LSCALE).astype(np.float16)
    return hi, lo


def _prep_shared(W1, b1, W2, b2):
    W1p = np.zeros((HP, D), np.float32)
    W1p[:H] = W1
    b1p = np.zeros((1, HP), np.float32)
    b1p[0, :H] = b1
    if FP16X3:
        W1p *= WSCALE
        b1p = b1p * WSCALE
    # w1T[m,p,k,j] = W1p[m*128+j, k*128+p]
    w1T = np.ascontiguousarray(
        W1p.reshape(M, 128, KT, 128).transpose(0, 3, 2, 1))
    W2p = np.zeros((A, HP), np.float32)
    W2p[:, :H] = W2
    # w2n[p, m*4+a] = -W2p[a, m*128+p]
    w2n = np.ascontiguousarray((-W2p).reshape(A, M, 128).transpose(2, 1, 0)
                               .reshape(128, M * A))
    s2b2 = (W2p.sum(axis=1, dtype=np.float32) + b2).reshape(1, A)
    s2b2 = np.ascontiguousarray(s2b2.astype(np.float32))
    shared = {"w2n": w2n, "s2b2": s2b2}
    if FP16X3:
        shared["w1h"], shared["w1l"] = _split16(w1T)
        shared["b1h"], shared["b1l"] = _split16(b1p)
    else:
        shared["w1h"] = w1T
        shared["b1h"] = b1p
    return shared


def _prep_x(x, c):
    # rows t-major: row = t*16 + b
    xs = np.ascontiguousarray(
        x[c * BL:(c + 1) * BL].transpose(1, 0, 2)).reshape(COLS, D)
    xT = np.ascontiguousarray(xs.T)                    # [D, COLS]
    # [128(p), KT, COLS]: xT3[p,k,c] = xT[k*128+p, c]
    xT3 = np.ascontiguousarray(xT.reshape(KT, 128, COLS).transpose(1, 0, 2))
    if FP16X3:
        hi, lo = _split16(xT3)
        return {"xh": hi, "xl": lo}
    return {"xh": xT3}


def kernel(x, W1, b1, W2, b2, _want_results=False):
    x = np.ascontiguousarray(np.asarray(x), np.float32)
    W1 = np.asarray(W1, np.float32)
    b1 = np.asarray(b1, np.float32)
    W2 = np.asarray(W2, np.float32)
    b2 = np.asarray(b2, np.float32)

    if "nc" not in _CACHE:
        _CACHE["nc"] = build()
    nc = _CACHE["nc"]

    shared = _prep_shared(W1, b1, W2, b2)
    in_maps = []
    for c in range(NCORES):
        m = dict(shared)
        m.update(_prep_x(x, c))
        in_maps.append(m)

    res = run_bass_kernel_spmd(nc, in_maps, core_ids=list(range(NCORES)))

    out = np.empty((B, T, A), np.float32)
    for c in range(NCORES):
        o = res.results[c]["out"]                      # [A, COLS], col=t*16+b
        out[c * BL:(c + 1) * BL] = o.T.reshape(T, BL, A).transpose(1, 0, 2)
    if _want_results:
        return out, res
    return out
